# revision 11
# baseline (speedup 1.0000x reference)
"""Sliding-window causal attention (window=1024) for B=2,T=2048,H=16,D=128 fp32
on 8 trn2 NeuronCores. Shards the 32 (batch, head) pairs 4-per-core.

Per (b,h): S^T = K @ Q^T blockwise (bf16 on the PE), exp on the scalar engine
writing bf16, then PV matmuls with the exp'd scores as the *stationary*
operand and [V | ones] as the moving operand: the
output lands as [q, d] with the softmax denominator in column 128 of the same
PSUM region, so no ones-matmul, no output transpose, and the normalize is a
per-partition scalar multiply. Q/K transposes run on the DMA XBAR.
"""
import math

import numpy as np

import concourse.bass as bass
import concourse.bacc as bacc
import concourse.mybir as mybir
from concourse import tile
from concourse.bass_utils import run_bass_kernel_spmd

B, T, H, D = 2, 2048, 16, 128
WINDOW = 1024
NCORES = 8
BH = B * H                  # 32 (b,h) pairs
BH_PER_CORE = BH // NCORES  # 4
NT = T // 128               # 16 seq tiles
G = 4                       # q-tiles per group (512 queries)
NG = NT // G
WB = WINDOW // 128          # window in blocks

f32 = mybir.dt.float32
bf16 = mybir.dt.bfloat16
f8e4 = mybir.dt.float8e4
AF = mybir.ActivationFunctionType
ALU = mybir.AluOpType
PM = mybir.MatmulPerfMode


def band_blocks(g):
    """Key blocks intersecting group g's sliding band, with the trimmed
    ABSOLUTE q-tile range [t_min, t_max] each block must serve."""
    out = []
    for b in range(max(0, G * g - WB), G * g + G):
        t_min = max(G * g, b)
        t_max = min(G * g + G - 1, b + WB)
        if t_min <= t_max:
            out.append((b, t_min, t_max))
    return out


def plan_group(g):
    """Greedy pairing of band blocks into DoubleRow pair segments plus
    leftover singles. Returns ops:
      ('pair', bA, t0, t1)  -- blocks bA, bA+1 both over q-tiles [t0, t1]
      ('single', b, t0, t1)
    """
    blocks = sorted(band_blocks(g), key=lambda x: (x[1], x[2]))
    ops = []
    i = 0
    while i < len(blocks):
        if i + 1 < len(blocks):
            bA, a0, a1 = blocks[i]
            bB, b0, b1 = blocks[i + 1]
            t0, t1 = max(a0, b0), min(a1, b1)
            if bB == bA + 1 and t0 <= t1:
                ops.append(('pair', bA, t0, t1))
                for (b, x0, x1) in ((bA, a0, a1), (bB, b0, b1)):
                    if x0 < t0:
                        ops.append(('single', b, x0, t0 - 1))
                    if x1 > t1:
                        ops.append(('single', b, t1 + 1, x1))
                i += 2
                continue
        b, x0, x1 = blocks[i]
        ops.append(('single', b, x0, x1))
        i += 1
    return ops


def build_nc(n_bh=BH_PER_CORE):
    nc = bacc.Bacc()
    q = nc.declare_dram_parameter("q", [n_bh, T, D], f32, isOutput=False)
    k = nc.declare_dram_parameter("k", [n_bh, T, D], f32, isOutput=False)
    v = nc.declare_dram_parameter("v", [n_bh, T, D], f32, isOutput=False)
    o = nc.declare_dram_parameter("o", [n_bh, T, D], f32, isOutput=True)

    scale = 1.0 / math.sqrt(D)

    with tile.TileContext(nc) as tc:
        with (
            tc.tile_pool(name="const", bufs=1) as constp,
            tc.tile_pool(name="io", bufs=2) as iop,
            tc.tile_pool(name="qt", bufs=2) as qtp,
            tc.tile_pool(name="es", bufs=10) as esp,
            tc.tile_pool(name="outp", bufs=3) as outp,
            tc.tile_pool(name="ps_st", bufs=2, space="PSUM") as ps_st,
            tc.tile_pool(name="ps_o", bufs=1, space="PSUM") as ps_o,
        ):
            # --- constants: diag / anti-diag keep-masks in fp8
            ones_f = constp.tile([128, 128], f32)
            mdiag_f = constp.tile([128, 128], f32)
            madiag_f = constp.tile([128, 128], f32)
            nc.gpsimd.memset(ones_f[:], 1.0)
            # diag mask (allowed k <= q): keep where col - p >= 0
            nc.gpsimd.affine_select(
                out=mdiag_f[:], in_=ones_f[:], compare_op=ALU.is_ge,
                fill=0.0, base=0, channel_multiplier=-1, pattern=[[1, 128]],
            )
            # anti-diag mask (allowed k > q): keep where p - col - 1 >= 0
            nc.gpsimd.affine_select(
                out=madiag_f[:], in_=ones_f[:], compare_op=ALU.is_ge,
                fill=0.0, base=-1, channel_multiplier=1, pattern=[[-1, 128]],
            )
            mdiag = constp.tile([128, 128], bf16)
            madiag = constp.tile([128, 128], bf16)
            nc.vector.tensor_copy(mdiag[:], mdiag_f[:])
            nc.vector.tensor_copy(madiag[:], madiag_f[:])

            # --- loads: fp32->bf16 (q,k) and fp32->fp8e4 (v) casts on SWDGE.
            def issue_loads(bh):
                qb = iop.tile([128, NT, 128], bf16, tag="qb", name=f"qb_{bh}")
                kb = iop.tile([128, NT, 128], bf16, tag="kb", name=f"kb_{bh}")
                v8 = iop.tile([128, NT, 130], bf16, tag="v8", name=f"v8_{bh}")
                nc.gpsimd.dma_start(
                    out=qb[:], in_=q[bh].rearrange("(n p) d -> p n d", p=128))
                nc.gpsimd.dma_start(
                    out=kb[:], in_=k[bh].rearrange("(n p) d -> p n d", p=128))
                nc.gpsimd.dma_start(
                    out=v8[:, :, 0:128],
                    in_=v[bh].rearrange("(n p) d -> p n d", p=128))
                nc.gpsimd.memset(v8[:, :, 128:129], 1.0)
                return qb, kb, v8

            def issue_transposes(bh, qb, kb):
                qt = qtp.tile([128, NT, 128], bf16, tag="qt", name=f"qt_{bh}")
                kt = qtp.tile([128, NT, 128], bf16, tag="kt", name=f"kt_{bh}")
                for n in range(NT):
                    nc.sync.dma_start(out=qt[:, n, :], in_=qb[:, n, :],
                                      transpose=True)
                    nc.sync.dma_start(out=kt[:, n, :], in_=kb[:, n, :],
                                      transpose=True)
                return qt, kt

            loaded = {0: issue_loads(0)}

            for bh in range(n_bh):
                qb, kb, v8 = loaded.pop(bh)
                qt, kt = issue_transposes(bh, qb, kb)
                if bh + 1 < n_bh:
                    loaded[bh + 1] = issue_loads(bh + 1)

                for g in range(NG):
                    ops = plan_group(g)
                    # per-tile contribution counts for start/stop bookkeeping
                    ncontrib = {t: 0 for t in range(G * g, G * g + G)}
                    for op in ops:
                        n = 2 if op[0] == 'pair' else 1
                        for t in range(op[2], op[3] + 1):
                            ncontrib[t] += n
                    seen = {t: 0 for t in ncontrib}

                    # O psum: one bank per q-tile; cols 0:128 = O, col 128 =
                    # softmax denominator
                    ot = [ps_o.tile([128, 512], f32, tag=f"ot{j}",
                                    name=f"ot{j}_{bh}_{g}")
                          for j in range(G)]

                    def o_region(t):
                        return ot[t - G * g][:, 0:129]

                    rcp = outp.tile([128, G], f32, tag="rcp")
                    oo = outp.tile([128, G, 128], f32, tag="oo")

                    def finish_tile(t):
                        i = t - G * g
                        reg = o_region(t)
                        nc.vector.reciprocal(rcp[:, i:i + 1], reg[:, 128:129])
                        nc.vector.tensor_scalar_mul(
                            oo[:, i, :], reg[:, 0:128], rcp[:, i:i + 1])

                    for op in ops:
                        if op[0] == 'pair':
                            _, bA, t0, t1 = op
                            w = (t1 - t0 + 1) * 128
                            st = ps_st.tile([128, 2, 512], f32, tag="st")
                            es = esp.tile([128, 2, 512], bf16, tag="es")
                            for s, b in ((0, bA), (1, bA + 1)):
                                nc.tensor.matmul(
                                    st[:, s, 0:w], kt[:, b, :],
                                    qt[:, t0:t1 + 1, :],
                                    start=True, stop=True)
                            nc.scalar.activation(
                                es[:, :, 0:w], st[:, :, 0:w], AF.Exp,
                                scale=scale)
                            for s, b in ((0, bA), (1, bA + 1)):
                                if t0 <= b <= t1:
                                    c = (b - t0) * 128
                                    nc.vector.tensor_mul(
                                        es[:, s, c:c + 128],
                                        es[:, s, c:c + 128], mdiag[:])
                                if t0 <= b + WB <= t1:
                                    c = (b + WB - t0) * 128
                                    nc.gpsimd.tensor_mul(
                                        es[:, s, c:c + 128],
                                        es[:, s, c:c + 128], madiag[:])
                            for t in range(t0, t1 + 1):
                                c = (t - t0) * 128
                                seen[t] += 2
                                nc.tensor.matmul(
                                    o_region(t), es[:, 0, c:c + 128],
                                    v8[:, bA, 0:129],
                                    start=(seen[t] == 2), stop=False)
                                nc.tensor.matmul(
                                    o_region(t), es[:, 1, c:c + 128],
                                    v8[:, bA + 1, 0:129],
                                    start=False,
                                    stop=(seen[t] == ncontrib[t]))
                                if seen[t] == ncontrib[t]:
                                    finish_tile(t)
                        else:
                            _, b, t0, t1 = op
                            w = (t1 - t0 + 1) * 128
                            st = ps_st.tile([128, 2, 512], f32, tag="st")
                            es = esp.tile([128, 2, 512], bf16, tag="es")
                            nc.tensor.matmul(
                                st[:, 0, 0:w], kt[:, b, :],
                                qt[:, t0:t1 + 1, :], start=True, stop=True)
                            nc.scalar.activation(
                                es[:, 0, 0:w], st[:, 0, 0:w], AF.Exp,
                                scale=scale)
                            if t0 <= b <= t1:
                                c = (b - t0) * 128
                                nc.vector.tensor_mul(
                                    es[:, 0, c:c + 128],
                                    es[:, 0, c:c + 128], mdiag[:])
                            if t0 <= b + WB <= t1:
                                c = (b + WB - t0) * 128
                                nc.gpsimd.tensor_mul(
                                    es[:, 0, c:c + 128],
                                    es[:, 0, c:c + 128], madiag[:])
                            for t in range(t0, t1 + 1):
                                c = (t - t0) * 128
                                seen[t] += 1
                                nc.tensor.matmul(
                                    o_region(t), es[:, 0, c:c + 128],
                                    v8[:, b, 0:129],
                                    start=(seen[t] == 1),
                                    stop=(seen[t] == ncontrib[t]))
                                if seen[t] == ncontrib[t]:
                                    finish_tile(t)

                    nc.sync.dma_start(
                        out=o[bh, 512 * g:512 * (g + 1), :].rearrange(
                            "(t p) d -> p t d", p=128),
                        in_=oo[:])
    if not nc.is_finalized():
        nc.finalize()
    return nc


_nc = None


def _get_nc():
    global _nc
    if _nc is None:
        _nc = build_nc()
    return _nc


def make_in_maps(q, k, v):
    q = np.ascontiguousarray(np.asarray(q, dtype=np.float32))
    k = np.ascontiguousarray(np.asarray(k, dtype=np.float32))
    v = np.ascontiguousarray(np.asarray(v, dtype=np.float32))
    # [B, T, H, D] -> [B*H, T, D]
    qs = np.ascontiguousarray(q.transpose(0, 2, 1, 3).reshape(BH, T, D))
    ks = np.ascontiguousarray(k.transpose(0, 2, 1, 3).reshape(BH, T, D))
    vs = np.ascontiguousarray(v.transpose(0, 2, 1, 3).reshape(BH, T, D))
    return [
        {
            "q": qs[c * BH_PER_CORE:(c + 1) * BH_PER_CORE],
            "k": ks[c * BH_PER_CORE:(c + 1) * BH_PER_CORE],
            "v": vs[c * BH_PER_CORE:(c + 1) * BH_PER_CORE],
        }
        for c in range(NCORES)
    ]


def assemble_out(results):
    out = np.empty((BH, T, D), np.float32)
    for c in range(NCORES):
        out[c * BH_PER_CORE:(c + 1) * BH_PER_CORE] = results[c]["o"]
    return np.ascontiguousarray(
        out.reshape(B, H, T, D).transpose(0, 2, 1, 3))


def kernel(q, k, v, window_size):
    assert int(window_size) == WINDOW
    in_maps = make_in_maps(q, k, v)
    res = run_bass_kernel_spmd(_get_nc(), in_maps, list(range(NCORES))).results
    return assemble_out(res)


# revision 13
# speedup vs baseline: 1.7418x; 1.7418x over previous
"""Sliding-window causal attention (window=1024) for B=2,T=2048,H=16,D=128 fp32
on 8 trn2 NeuronCores. Shards the 32 (batch, head) pairs 4-per-core.

Per (b,h): S^T = K @ Q^T blockwise (bf16 on the PE), exp on the scalar engine
writing bf16, then PV matmuls with the exp'd scores as the *stationary*
operand and [V | ones] as the moving operand: the
output lands as [q, d] with the softmax denominator in column 128 of the same
PSUM region, so no ones-matmul, no output transpose, and the normalize is a
per-partition scalar multiply.
"""
import math

import numpy as np

import concourse.bass as bass
import concourse.bacc as bacc
import concourse.mybir as mybir
from concourse import tile
from concourse.bass_utils import run_bass_kernel_spmd

B, T, H, D = 2, 2048, 16, 128
WINDOW = 1024
NCORES = 8
BH = B * H                  # 32 (b,h) pairs
BH_PER_CORE = BH // NCORES  # 4
NT = T // 128               # 16 seq tiles
G = 4                       # q-tiles per group (512 queries)
NG = NT // G
WB = WINDOW // 128          # window in blocks

f32 = mybir.dt.float32
bf16 = mybir.dt.bfloat16
f8e4 = mybir.dt.float8e4
AF = mybir.ActivationFunctionType
ALU = mybir.AluOpType
PM = mybir.MatmulPerfMode


def band_blocks(g):
    """Key blocks intersecting group g's sliding band, with the trimmed
    ABSOLUTE q-tile range [t_min, t_max] each block must serve."""
    out = []
    for b in range(max(0, G * g - WB), G * g + G):
        t_min = max(G * g, b)
        t_max = min(G * g + G - 1, b + WB)
        if t_min <= t_max:
            out.append((b, t_min, t_max))
    return out


def plan_group(g):
    """Greedy pairing of band blocks into DoubleRow pair segments plus
    leftover singles. Returns ops:
      ('pair', bA, t0, t1)  -- blocks bA, bA+1 both over q-tiles [t0, t1]
      ('single', b, t0, t1)
    """
    blocks = sorted(band_blocks(g), key=lambda x: (x[1], x[2]))
    ops = []
    i = 0
    while i < len(blocks):
        if i + 1 < len(blocks):
            bA, a0, a1 = blocks[i]
            bB, b0, b1 = blocks[i + 1]
            t0, t1 = max(a0, b0), min(a1, b1)
            if bB == bA + 1 and t0 <= t1:
                ops.append(('pair', bA, t0, t1))
                for (b, x0, x1) in ((bA, a0, a1), (bB, b0, b1)):
                    if x0 < t0:
                        ops.append(('single', b, x0, t0 - 1))
                    if x1 > t1:
                        ops.append(('single', b, t1 + 1, x1))
                i += 2
                continue
        b, x0, x1 = blocks[i]
        ops.append(('single', b, x0, x1))
        i += 1
    return ops


def build_nc(n_bh=BH_PER_CORE):
    nc = bacc.Bacc()
    q = nc.declare_dram_parameter("q", [n_bh, T, D], f32, isOutput=False)
    k = nc.declare_dram_parameter("k", [n_bh, T, D], f32, isOutput=False)
    v = nc.declare_dram_parameter("v", [n_bh, T, D], f32, isOutput=False)
    o = nc.declare_dram_parameter("o", [n_bh, T, D], f32, isOutput=True)

    scale = 1.0 / math.sqrt(D)

    with tile.TileContext(nc) as tc:
        with (
            tc.tile_pool(name="const", bufs=1) as constp,
            tc.tile_pool(name="io", bufs=2) as iop,
            tc.tile_pool(name="qt", bufs=2) as qtp,
            tc.tile_pool(name="es", bufs=10) as esp,
            tc.tile_pool(name="outp", bufs=3) as outp,
            tc.tile_pool(name="ps_st", bufs=2, space="PSUM") as ps_st,
            tc.tile_pool(name="ps_o", bufs=1, space="PSUM") as ps_o,
        ):
            # --- constants: diag / anti-diag keep-masks in fp8
            ones_f = constp.tile([128, 128], f32)
            mdiag_f = constp.tile([128, 128], f32)
            madiag_f = constp.tile([128, 128], f32)
            nc.gpsimd.memset(ones_f[:], 1.0)
            # diag mask (allowed k <= q): keep where col - p >= 0
            nc.gpsimd.affine_select(
                out=mdiag_f[:], in_=ones_f[:], compare_op=ALU.is_ge,
                fill=0.0, base=0, channel_multiplier=-1, pattern=[[1, 128]],
            )
            # anti-diag mask (allowed k > q): keep where p - col - 1 >= 0
            nc.gpsimd.affine_select(
                out=madiag_f[:], in_=ones_f[:], compare_op=ALU.is_ge,
                fill=0.0, base=-1, channel_multiplier=1, pattern=[[-1, 128]],
            )
            ident_f = constp.tile([128, 128], f32)
            nc.gpsimd.affine_select(
                out=ident_f[:], in_=ones_f[:], compare_op=ALU.is_equal,
                fill=0.0, base=0, channel_multiplier=1, pattern=[[-1, 128]],
            )
            mdiag = constp.tile([128, 128], bf16)
            madiag = constp.tile([128, 128], bf16)
            ident = constp.tile([128, 128], bf16)
            nc.vector.tensor_copy(mdiag[:], mdiag_f[:])
            nc.vector.tensor_copy(madiag[:], madiag_f[:])
            nc.vector.tensor_copy(ident[:], ident_f[:])

            # --- loads: fp32->bf16 (q,k) and fp32->fp8e4 (v) casts on SWDGE.
            def issue_loads(bh):
                qb = iop.tile([128, NT, 128], bf16, tag="qb", name=f"qb_{bh}")
                kb = iop.tile([128, NT, 128], bf16, tag="kb", name=f"kb_{bh}")
                v8 = iop.tile([128, NT, 130], bf16, tag="v8", name=f"v8_{bh}")
                nc.gpsimd.dma_start(
                    out=qb[:], in_=q[bh].rearrange("(n p) d -> p n d", p=128))
                nc.gpsimd.dma_start(
                    out=kb[:], in_=k[bh].rearrange("(n p) d -> p n d", p=128))
                nc.gpsimd.dma_start(
                    out=v8[:, :, 0:128],
                    in_=v[bh].rearrange("(n p) d -> p n d", p=128))
                nc.gpsimd.memset(v8[:, :, 128:129], 1.0)
                return qb, kb, v8

            def issue_transposes(bh, qb, kb):
                qt = qtp.tile([128, NT, 128], bf16, tag="qt", name=f"qt_{bh}")
                kt = qtp.tile([128, NT, 128], bf16, tag="kt", name=f"kt_{bh}")
                for quad in range(NT // 4):
                    for src_t, dst in ((qb, qt), (kb, kt)):
                        trt = ps_st.tile([128, 2, 512], f32, tag="st",
                                         name=f"tr_{bh}_{quad}")
                        tr = trt.bitcast(bf16).rearrange("p a b -> p (a b)")
                        for i in range(4):
                            n = quad * 4 + i
                            nc.tensor.matmul(
                                tr[:, i * 128:(i + 1) * 128],
                                src_t[:, n, :], ident[:],
                                is_transpose=True,
                                start=(i == 0), stop=(i == 3))
                        nc.vector.tensor_copy(
                            dst[:, quad * 4:quad * 4 + 4, :], tr[:, 0:512])
                return qt, kt

            loaded = {0: issue_loads(0)}

            for bh in range(n_bh):
                qb, kb, v8 = loaded.pop(bh)
                qt, kt = issue_transposes(bh, qb, kb)
                if bh + 1 < n_bh:
                    loaded[bh + 1] = issue_loads(bh + 1)

                for g in range(NG):
                    ops = plan_group(g)
                    # per-tile contribution counts for start/stop bookkeeping
                    ncontrib = {t: 0 for t in range(G * g, G * g + G)}
                    for op in ops:
                        n = 2 if op[0] == 'pair' else 1
                        for t in range(op[2], op[3] + 1):
                            ncontrib[t] += n
                    seen = {t: 0 for t in ncontrib}

                    # O psum: one bank per q-tile; cols 0:128 = O, col 128
                    # = softmax denominator
                    ot = [ps_o.tile([128, 512], f32, tag=f"ot{j}",
                                    name=f"ot{j}_{bh}_{g}")
                          for j in range(G)]

                    def o_region(t):
                        return ot[t - G * g][:, 0:129]

                    rcp = outp.tile([128, G], f32, tag="rcp")
                    oo = outp.tile([128, G, 128], f32, tag="oo")

                    def finish_tile(t):
                        i = t - G * g
                        reg = o_region(t)
                        nc.vector.reciprocal(rcp[:, i:i + 1], reg[:, 128:129])
                        nc.vector.tensor_scalar_mul(
                            oo[:, i, :], reg[:, 0:128], rcp[:, i:i + 1])

                    for op in ops:
                        if op[0] == 'pair':
                            _, bA, t0, t1 = op
                            w = (t1 - t0 + 1) * 128
                            st = ps_st.tile([128, 2, 512], f32, tag="st")
                            es = esp.tile([128, 2, 512], bf16, tag="es")
                            for s, b in ((0, bA), (1, bA + 1)):
                                nc.tensor.matmul(
                                    st[:, s, 0:w], kt[:, b, :],
                                    qt[:, t0:t1 + 1, :],
                                    start=True, stop=True)
                            nc.scalar.activation(
                                es[:, :, 0:w], st[:, :, 0:w], AF.Exp,
                                scale=scale)
                            for s, b in ((0, bA), (1, bA + 1)):
                                if t0 <= b <= t1:
                                    c = (b - t0) * 128
                                    nc.vector.tensor_mul(
                                        es[:, s, c:c + 128],
                                        es[:, s, c:c + 128], mdiag[:])
                                if t0 <= b + WB <= t1:
                                    c = (b + WB - t0) * 128
                                    nc.gpsimd.tensor_mul(
                                        es[:, s, c:c + 128],
                                        es[:, s, c:c + 128], madiag[:])
                            for t in range(t0, t1 + 1):
                                c = (t - t0) * 128
                                seen[t] += 2
                                nc.tensor.matmul(
                                    o_region(t), es[:, 0, c:c + 128],
                                    v8[:, bA, 0:129],
                                    start=(seen[t] == 2), stop=False)
                                nc.tensor.matmul(
                                    o_region(t), es[:, 1, c:c + 128],
                                    v8[:, bA + 1, 0:129],
                                    start=False,
                                    stop=(seen[t] == ncontrib[t]))
                                if seen[t] == ncontrib[t]:
                                    finish_tile(t)
                        else:
                            _, b, t0, t1 = op
                            w = (t1 - t0 + 1) * 128
                            st = ps_st.tile([128, 2, 512], f32, tag="st")
                            es = esp.tile([128, 2, 512], bf16, tag="es")
                            nc.tensor.matmul(
                                st[:, 0, 0:w], kt[:, b, :],
                                qt[:, t0:t1 + 1, :], start=True, stop=True)
                            nc.scalar.activation(
                                es[:, 0, 0:w], st[:, 0, 0:w], AF.Exp,
                                scale=scale)
                            if t0 <= b <= t1:
                                c = (b - t0) * 128
                                nc.vector.tensor_mul(
                                    es[:, 0, c:c + 128],
                                    es[:, 0, c:c + 128], mdiag[:])
                            if t0 <= b + WB <= t1:
                                c = (b + WB - t0) * 128
                                nc.gpsimd.tensor_mul(
                                    es[:, 0, c:c + 128],
                                    es[:, 0, c:c + 128], madiag[:])
                            for t in range(t0, t1 + 1):
                                c = (t - t0) * 128
                                seen[t] += 1
                                nc.tensor.matmul(
                                    o_region(t), es[:, 0, c:c + 128],
                                    v8[:, b, 0:129],
                                    start=(seen[t] == 1),
                                    stop=(seen[t] == ncontrib[t]))
                                if seen[t] == ncontrib[t]:
                                    finish_tile(t)

                    nc.sync.dma_start(
                        out=o[bh, 512 * g:512 * (g + 1), :].rearrange(
                            "(t p) d -> p t d", p=128),
                        in_=oo[:])
    if not nc.is_finalized():
        nc.finalize()
    return nc


_nc = None


def _get_nc():
    global _nc
    if _nc is None:
        _nc = build_nc()
    return _nc


def make_in_maps(q, k, v):
    q = np.ascontiguousarray(np.asarray(q, dtype=np.float32))
    k = np.ascontiguousarray(np.asarray(k, dtype=np.float32))
    v = np.ascontiguousarray(np.asarray(v, dtype=np.float32))
    # [B, T, H, D] -> [B*H, T, D]
    qs = np.ascontiguousarray(q.transpose(0, 2, 1, 3).reshape(BH, T, D))
    ks = np.ascontiguousarray(k.transpose(0, 2, 1, 3).reshape(BH, T, D))
    vs = np.ascontiguousarray(v.transpose(0, 2, 1, 3).reshape(BH, T, D))
    return [
        {
            "q": qs[c * BH_PER_CORE:(c + 1) * BH_PER_CORE],
            "k": ks[c * BH_PER_CORE:(c + 1) * BH_PER_CORE],
            "v": vs[c * BH_PER_CORE:(c + 1) * BH_PER_CORE],
        }
        for c in range(NCORES)
    ]


def assemble_out(results):
    out = np.empty((BH, T, D), np.float32)
    for c in range(NCORES):
        out[c * BH_PER_CORE:(c + 1) * BH_PER_CORE] = results[c]["o"]
    return np.ascontiguousarray(
        out.reshape(B, H, T, D).transpose(0, 2, 1, 3))


def kernel(q, k, v, window_size):
    assert int(window_size) == WINDOW
    in_maps = make_in_maps(q, k, v)
    res = run_bass_kernel_spmd(_get_nc(), in_maps, list(range(NCORES))).results
    return assemble_out(res)


# revision 14
# speedup vs baseline: 2.9274x; 1.6807x over previous
"""Sliding-window causal attention (window=1024) for B=2,T=2048,H=16,D=128 fp32
on 8 trn2 NeuronCores. Shards the 32 (batch, head) pairs 4-per-core.

Per (b,h): S^T = K @ Q^T blockwise (bf16 on the PE), exp on the scalar engine
writing bf16, then PV matmuls with the exp'd scores as the *stationary*
operand and [V | ones] as the moving operand: the
output lands as [q, d] with the softmax denominator in column 128 of the same
PSUM region, so no ones-matmul, no output transpose, and the normalize is a
per-partition scalar multiply.
"""
import math

import numpy as np

import concourse.bass as bass
import concourse.bacc as bacc
import concourse.mybir as mybir
from concourse import tile
from concourse.bass_utils import run_bass_kernel_spmd

B, T, H, D = 2, 2048, 16, 128
WINDOW = 1024
NCORES = 8
BH = B * H                  # 32 (b,h) pairs
BH_PER_CORE = BH // NCORES  # 4
NT = T // 128               # 16 seq tiles
G = 4                       # q-tiles per group (512 queries)
NG = NT // G
WB = WINDOW // 128          # window in blocks

f32 = mybir.dt.float32
bf16 = mybir.dt.bfloat16
f8e4 = mybir.dt.float8e4
AF = mybir.ActivationFunctionType
ALU = mybir.AluOpType
PM = mybir.MatmulPerfMode


def band_blocks(g):
    """Key blocks intersecting group g's sliding band, with the trimmed
    ABSOLUTE q-tile range [t_min, t_max] each block must serve."""
    out = []
    for b in range(max(0, G * g - WB), G * g + G):
        t_min = max(G * g, b)
        t_max = min(G * g + G - 1, b + WB)
        if t_min <= t_max:
            out.append((b, t_min, t_max))
    return out


def plan_group_packed(g):
    """Pack the band's (block, q-tile-range) segments densely into PSUM
    banks of 4 tile-columns (512 f32). Returns a list of banks, each a
    list of segments (b, t0, t1, col_off)."""
    banks, cur, used = [], [], 0
    for (b, t0, t1) in band_blocks(g):
        while t0 <= t1:
            take = min(4 - used, t1 - t0 + 1)
            cur.append((b, t0, t0 + take - 1, used * 128))
            used += take
            t0 += take
            if used == 4:
                banks.append(cur)
                cur, used = [], 0
    if cur:
        banks.append(cur)
    return banks


def build_nc(n_bh=BH_PER_CORE):
    nc = bacc.Bacc()
    q = nc.declare_dram_parameter("q", [n_bh, T, D], f32, isOutput=False)
    k = nc.declare_dram_parameter("k", [n_bh, T, D], f32, isOutput=False)
    v = nc.declare_dram_parameter("v", [n_bh, T, D], f32, isOutput=False)
    o = nc.declare_dram_parameter("o", [n_bh, T, D], f32, isOutput=True)

    scale = 1.0 / math.sqrt(D)

    with tile.TileContext(nc) as tc:
        with (
            tc.tile_pool(name="const", bufs=1) as constp,
            tc.tile_pool(name="io", bufs=2) as iop,
            tc.tile_pool(name="qt", bufs=2) as qtp,
            tc.tile_pool(name="es", bufs=10) as esp,
            tc.tile_pool(name="outp", bufs=3) as outp,
            tc.tile_pool(name="ps_st", bufs=4, space="PSUM") as ps_st,
            tc.tile_pool(name="ps_o", bufs=1, space="PSUM") as ps_o,
        ):
            # --- constants: diag / anti-diag keep-masks in fp8
            ones_f = constp.tile([128, 128], f32)
            mdiag_f = constp.tile([128, 128], f32)
            madiag_f = constp.tile([128, 128], f32)
            nc.gpsimd.memset(ones_f[:], 1.0)
            # diag mask (allowed k <= q): keep where col - p >= 0
            nc.gpsimd.affine_select(
                out=mdiag_f[:], in_=ones_f[:], compare_op=ALU.is_ge,
                fill=0.0, base=0, channel_multiplier=-1, pattern=[[1, 128]],
            )
            # anti-diag mask (allowed k > q): keep where p - col - 1 >= 0
            nc.gpsimd.affine_select(
                out=madiag_f[:], in_=ones_f[:], compare_op=ALU.is_ge,
                fill=0.0, base=-1, channel_multiplier=1, pattern=[[-1, 128]],
            )
            ident_f = constp.tile([128, 128], f32)
            nc.gpsimd.affine_select(
                out=ident_f[:], in_=ones_f[:], compare_op=ALU.is_equal,
                fill=0.0, base=0, channel_multiplier=1, pattern=[[-1, 128]],
            )
            mdiag = constp.tile([128, 128], bf16)
            madiag = constp.tile([128, 128], bf16)
            ident = constp.tile([128, 128], bf16)
            nc.vector.tensor_copy(mdiag[:], mdiag_f[:])
            nc.vector.tensor_copy(madiag[:], madiag_f[:])
            nc.vector.tensor_copy(ident[:], ident_f[:])

            # --- loads: fp32->bf16 (q,k) and fp32->fp8e4 (v) casts on SWDGE.
            def issue_loads(bh):
                qb = iop.tile([128, NT, 128], bf16, tag="qb", name=f"qb_{bh}")
                kb = iop.tile([128, NT, 128], bf16, tag="kb", name=f"kb_{bh}")
                v8 = iop.tile([128, NT, 130], bf16, tag="v8", name=f"v8_{bh}")
                nc.gpsimd.dma_start(
                    out=qb[:], in_=q[bh].rearrange("(n p) d -> p n d", p=128))
                nc.gpsimd.dma_start(
                    out=kb[:], in_=k[bh].rearrange("(n p) d -> p n d", p=128))
                nc.gpsimd.dma_start(
                    out=v8[:, :, 0:128],
                    in_=v[bh].rearrange("(n p) d -> p n d", p=128))
                nc.gpsimd.memset(v8[:, :, 128:129], 1.0)
                return qb, kb, v8

            def issue_transposes(bh, qb, kb):
                qt = qtp.tile([128, NT, 128], bf16, tag="qt", name=f"qt_{bh}")
                kt = qtp.tile([128, NT, 128], bf16, tag="kt", name=f"kt_{bh}")
                for quad in range(NT // 4):
                    for src_t, dst in ((qb, qt), (kb, kt)):
                        trt = ps_st.tile([128, 512], f32, tag="st",
                                         name=f"tr_{bh}_{quad}")
                        tr = trt.bitcast(bf16)
                        for i in range(4):
                            n = quad * 4 + i
                            nc.tensor.matmul(
                                tr[:, i * 128:(i + 1) * 128],
                                src_t[:, n, :], ident[:],
                                is_transpose=True,
                                start=(i == 0), stop=(i == 3),
                                skip_group_check=True)
                        nc.vector.tensor_copy(
                            dst[:, quad * 4:quad * 4 + 4, :], tr[:, 0:512])
                return qt, kt

            loaded = {0: issue_loads(0)}

            for bh in range(n_bh):
                qb, kb, v8 = loaded.pop(bh)
                qt, kt = issue_transposes(bh, qb, kb)
                if bh + 1 < n_bh:
                    loaded[bh + 1] = issue_loads(bh + 1)

                for g in range(NG):
                    banks = plan_group_packed(g)
                    ncontrib = {t: 0 for t in range(G * g, G * g + G)}
                    for segs in banks:
                        for (b, t0, t1, off) in segs:
                            for t in range(t0, t1 + 1):
                                ncontrib[t] += 1
                    seen = {t: 0 for t in ncontrib}

                    # O psum: one bank per q-tile; cols 0:128 = O, col 128
                    # = softmax denominator
                    ot = [ps_o.tile([128, 512], f32, tag=f"ot{j}",
                                    name=f"ot{j}_{bh}_{g}")
                          for j in range(G)]

                    def o_region(t):
                        return ot[t - G * g][:, 0:129]

                    rcp = outp.tile([128, G], f32, tag="rcp")
                    oo = outp.tile([128, G, 128], f32, tag="oo")

                    def finish_tile(t):
                        i = t - G * g
                        reg = o_region(t)
                        nc.vector.reciprocal(rcp[:, i:i + 1], reg[:, 128:129])
                        nc.vector.tensor_scalar_mul(
                            oo[:, i, :], reg[:, 0:128], rcp[:, i:i + 1])

                    for segs in banks:
                        wtot = sum((t1 - t0 + 1) for (_, t0, t1, _) in segs)
                        st = ps_st.tile([128, 512], f32, tag="st")
                        es = esp.tile([128, 512], bf16, tag="es")
                        for (b, t0, t1, off) in segs:
                            w = (t1 - t0 + 1) * 128
                            nc.tensor.matmul(
                                st[:, off:off + w], kt[:, b, :],
                                qt[:, t0:t1 + 1, :],
                                start=True, stop=True,
                                skip_group_check=True)
                        nc.scalar.activation(
                            es[:, 0:wtot * 128], st[:, 0:wtot * 128],
                            AF.Exp, scale=scale)
                        for (b, t0, t1, off) in segs:
                            if t0 <= b <= t1:
                                c = off + (b - t0) * 128
                                nc.vector.tensor_mul(
                                    es[:, c:c + 128], es[:, c:c + 128],
                                    mdiag[:])
                            if t0 <= b + WB <= t1:
                                c = off + (b + WB - t0) * 128
                                nc.gpsimd.tensor_mul(
                                    es[:, c:c + 128], es[:, c:c + 128],
                                    madiag[:])
                        for (b, t0, t1, off) in segs:
                            for t in range(t0, t1 + 1):
                                c = off + (t - t0) * 128
                                seen[t] += 1
                                nc.tensor.matmul(
                                    o_region(t), es[:, c:c + 128],
                                    v8[:, b, 0:129],
                                    start=(seen[t] == 1),
                                    stop=(seen[t] == ncontrib[t]))
                                if seen[t] == ncontrib[t]:
                                    finish_tile(t)

                    nc.sync.dma_start(
                        out=o[bh, 512 * g:512 * (g + 1), :].rearrange(
                            "(t p) d -> p t d", p=128),
                        in_=oo[:])
    if not nc.is_finalized():
        nc.finalize()
    return nc


_nc = None


def _get_nc():
    global _nc
    if _nc is None:
        _nc = build_nc()
    return _nc


def make_in_maps(q, k, v):
    q = np.ascontiguousarray(np.asarray(q, dtype=np.float32))
    k = np.ascontiguousarray(np.asarray(k, dtype=np.float32))
    v = np.ascontiguousarray(np.asarray(v, dtype=np.float32))
    # [B, T, H, D] -> [B*H, T, D]
    qs = np.ascontiguousarray(q.transpose(0, 2, 1, 3).reshape(BH, T, D))
    ks = np.ascontiguousarray(k.transpose(0, 2, 1, 3).reshape(BH, T, D))
    vs = np.ascontiguousarray(v.transpose(0, 2, 1, 3).reshape(BH, T, D))
    return [
        {
            "q": qs[c * BH_PER_CORE:(c + 1) * BH_PER_CORE],
            "k": ks[c * BH_PER_CORE:(c + 1) * BH_PER_CORE],
            "v": vs[c * BH_PER_CORE:(c + 1) * BH_PER_CORE],
        }
        for c in range(NCORES)
    ]


def assemble_out(results):
    out = np.empty((BH, T, D), np.float32)
    for c in range(NCORES):
        out[c * BH_PER_CORE:(c + 1) * BH_PER_CORE] = results[c]["o"]
    return np.ascontiguousarray(
        out.reshape(B, H, T, D).transpose(0, 2, 1, 3))


def kernel(q, k, v, window_size):
    assert int(window_size) == WINDOW
    in_maps = make_in_maps(q, k, v)
    res = run_bass_kernel_spmd(_get_nc(), in_maps, list(range(NCORES))).results
    return assemble_out(res)
